# revision 27
# baseline (speedup 1.0000x reference)
"""Trainium2 Bass kernel for a basis-customized linear layer.

Reference computation (B=1024, IN=OUT=512, EMB=64, KQ=64, NB=3, VOCAB=100):
    embs = concat(emb_author[idx_author], emb_citation[idx_citation])  # [B, 128]
    h    = tanh(embs @ W1.T + b1)                                      # [B, 64]
    coef = softmax(h @ W2.T)                                           # [B, 3]
    w    = (coef @ W3.T + b3).reshape(B, IN, OUT)
    out  = einsum('bi,bio->bo', x, w)                                  # [B, 512]

Key rewrites:
  (1) w[b] = sum_j coef[b,j]*W3j + b3r, so
      out = sum_j coef[:,j] * (x @ W3j) + x @ b3r
      -- 3 shared [512,512] matmuls + a per-sample weighted combine, instead
      of materializing the 1GB per-sample weight.
  (2) since softmax coefs sum to 1, the bias folds into every basis block:
      out = sum_j coef[:,j] * (x @ (W3j + b3r))
  (3) the embedding gather is a one-hot matmul (the one-hot comes from a
      DMA-broadcast idx compared against an iota), fused with W1 via the
      host-precomputed per-vocab table G = emb @ W1half.T (param-only fold).

Sharding over 8 cores: batch 2-way x out-column 4-way. Each core gets
x.T[:, batch_half] (1MB) and its 3 basis column-blocks (0.75MB), computes
coef for its 512 rows on-device, and writes a [512, 128] output block.

Data movement: all big loads ride ONE queue (same-queue transfers complete in
order) as (w_k, x_k) pairs so the k-th accumulation sweep starts as soon as
its pair lands; the per-m combines then stagger behind the last sweep.
"""

import numpy as np
import ml_dtypes

import concourse.bass as bass
import concourse.tile as tile
from concourse import bacc, mybir
from concourse.bass_utils import run_bass_kernel_spmd

# Problem dims (hardcoded per contract)
B, IN, OUT = 1024, 512, 512
EMB, KQ, NB, VOCAB = 64, 64, 3, 100
P_B, Q_O = 4, 2            # batch shards x out-col shards = 8 cores
BS = B // P_B              # 256 batch rows per core
OS = 128                   # out-col block width
OB = OUT // Q_O // OS      # 2 out-col blocks per core
KT = IN // 128             # 4 contraction tiles
MT = BS // 128             # 2 batch tiles per core

F32 = mybir.dt.float32
F32R = mybir.dt.float32r
BF16 = mybir.dt.bfloat16

SF32 = NB + 1              # packed f32 small-input columns (W2.T | b1)
SB16 = 2 * BS + VOCAB      # idx_author | idx_citation | -iota rows, bf16
G2 = 2 * KQ                # gather-table columns at the head of wc

LAST_RESULT = None         # BassKernelResults of the most recent run (for test.py)

_NC_CACHE = None


def _ensure_ntff_hook_module():
    """bass_utils imports antenv.axon_hooks when BASS_TRACE is set; the module
    is absent on this image. Provide a no-op shim so tracing degrades
    gracefully instead of crashing."""
    import sys, types
    if "antenv.axon_hooks" in sys.modules:
        return
    try:
        import antenv
        import antenv.axon_hooks  # noqa: F401
    except ImportError:
        mod = types.ModuleType("antenv.axon_hooks")
        state = {"hook": None}
        mod.set_axon_ntff_profile_hook = lambda h: state.__setitem__("hook", h)
        mod.get_axon_ntff_profile_hook = lambda: state["hook"]
        sys.modules["antenv.axon_hooks"] = mod
        try:
            antenv.axon_hooks = mod
        except Exception:
            pass


def _build_nc():
    nc = bacc.Bacc("TRN2", target_bir_lowering=False, debug=False,
                   num_devices=P_B * Q_O)

    xt = nc.dram_tensor("xt", [128, KT * BS], F32R, kind="ExternalInput")
    wc = nc.dram_tensor("wc", [128, G2 + KT * OB * NB * OS], F32R,
                        kind="ExternalInput")
    sf = nc.dram_tensor("sf", [KQ, SF32], F32, kind="ExternalInput")
    sb = nc.dram_tensor("sb", [2, SB16], BF16, kind="ExternalInput")
    out = nc.dram_tensor("out", [128, MT * OB * OS], F32,
                         kind="ExternalOutput")

    with tile.TileContext(nc) as tc:
        with (
            tc.tile_pool(name="consts", bufs=1) as consts,
            tc.tile_pool(name="work", bufs=3) as work,
            tc.tile_pool(name="ps_idx", bufs=2, space="PSUM") as ps_idx,
            tc.tile_pool(name="ps_pre", bufs=2, space="PSUM") as ps_pre,
            tc.tile_pool(name="ps_y", bufs=4, space="PSUM") as ps_y,
        ):
            # ---- small loads on idle queues ----
            sb_sb = consts.tile([2, SB16], BF16)
            nc.scalar.dma_start(out=sb_sb, in_=sb[:, :])
            sf_sb = consts.tile([KQ, SF32], F32)
            nc.scalar.dma_start(out=sf_sb, in_=sf[:, :])
            idxa_sb = sb_sb[:, 0:BS]
            idxc_sb = sb_sb[:, BS:2 * BS]
            bw_sb = sb_sb[:, 2 * BS:2 * BS + VOCAB]

            # ---- big loads: ONE queue. ob0's weights ride k-paired with x
            # so its Y groups + combines finish while ob1's weights (one
            # trailing chunk) are still streaming in.
            WKO = NB * OS
            xall = consts.tile([128, KT, BS], F32R)
            wall = consts.tile([128, G2 + OB * KT * WKO], F32R)
            xt_k = xt[:, :].rearrange("p (k n) -> p k n", k=KT)
            nc.sync.dma_start(out=wall[:, 0:G2 + WKO],
                              in_=wc[:, 0:G2 + WKO])
            nc.sync.dma_start(out=xall[:, 0, :], in_=xt_k[:, 0, :])
            for k in range(1, KT):
                sl = slice(G2 + k * WKO, G2 + (k + 1) * WKO)
                nc.sync.dma_start(out=wall[:, sl], in_=wc[:, sl])
                nc.sync.dma_start(out=xall[:, k, :], in_=xt_k[:, k, :])
            sl1 = slice(G2 + KT * WKO, G2 + 2 * KT * WKO)
            nc.sync.dma_start(out=wall[:, sl1], in_=wc[:, sl1])

            gat_sb = wall[0:VOCAB, 0:KQ]
            gct_sb = wall[0:VOCAB, KQ:G2]

            # b1 routed through ACT so Tanh's bias dep is same-engine
            b1_sb = consts.tile([KQ, 1], F32)
            nc.scalar.copy(out=b1_sb, in_=sf_sb[:, NB:NB + 1])
            w2r_sb = consts.tile([KQ, NB + 1], F32R)
            nc.vector.memset(w2r_sb.bitcast(mybir.dt.uint32), 0)
            nc.vector.tensor_copy(out=w2r_sb[:, 0:NB], in_=sf_sb[:, 0:NB])

            # ---- stage A: coef for all BS rows ----
            # one-hot via K=2 matmul: psum[v,b] = idx[b]*1 + 1*(-v), then ==0
            bca_ps = ps_idx.tile([VOCAB, BS], F32, tag="idx")
            nc.tensor.matmul(bca_ps, lhsT=bw_sb, rhs=idxa_sb, start=True, stop=True)
            oha_sb = consts.tile([VOCAB, BS], F32R)
            nc.vector.tensor_scalar(
                out=oha_sb, in0=bca_ps, scalar1=0.0, scalar2=None,
                op0=mybir.AluOpType.is_equal,
            )
            bcc_ps = ps_idx.tile([VOCAB, BS], F32, tag="idx")
            nc.tensor.matmul(bcc_ps, lhsT=bw_sb, rhs=idxc_sb, start=True, stop=True)
            ohc_sb = consts.tile([VOCAB, BS], F32R)
            nc.vector.tensor_scalar(
                out=ohc_sb, in0=bcc_ps, scalar1=0.0, scalar2=None,
                op0=mybir.AluOpType.is_equal,
            )

            # fused gather + W1: preact.T [KQ, BS]
            pre_ps = ps_pre.tile([KQ, BS], F32, tag="pre")
            nc.tensor.matmul(pre_ps, lhsT=gat_sb, rhs=oha_sb, start=True, stop=False)
            nc.tensor.matmul(pre_ps, lhsT=gct_sb, rhs=ohc_sb, start=False, stop=True)
            ht_sb = consts.tile([KQ, BS], F32R)
            nc.scalar.activation(
                out=ht_sb, in_=pre_ps, func=mybir.ActivationFunctionType.Tanh,
                bias=b1_sb, scale=1.0,
            )

            # softmax over NB bases, per 128-row tile
            coef_sb = []
            for m in range(MT):
                lg_ps = ps_pre.tile([128, NB + 1], F32, tag="pre")
                nc.tensor.matmul(
                    lg_ps, lhsT=ht_sb[:, m * 128:(m + 1) * 128], rhs=w2r_sb,
                    start=True, stop=True,
                )
                e_sb = work.tile([128, NB], F32, tag="e")
                nc.scalar.activation(
                    out=e_sb, in_=lg_ps[:, 0:NB],
                    func=mybir.ActivationFunctionType.Exp,
                )
                s_sb = work.tile([128, 1], F32, tag="s")
                nc.vector.reduce_sum(out=s_sb, in_=e_sb, axis=mybir.AxisListType.X)
                r_sb = work.tile([128, 1], F32, tag="r")
                nc.vector.reciprocal(out=r_sb, in_=s_sb)
                cf = consts.tile([128, NB], F32, name=f"coef{m}", tag=f"coef{m}")
                nc.vector.tensor_scalar(
                    out=cf, in0=e_sb, scalar1=r_sb, scalar2=None,
                    op0=mybir.AluOpType.mult,
                )
                coef_sb.append(cf)

            # ---- stage B + combine, ob-outer: ob0 finishes and combines
            # while ob1's weights stream
            y_ps = [
                [ps_y.tile([128, OS, NB], F32, tag="y", name=f"y{m}_{ob}")
                 for ob in range(OB)]
                for m in range(MT)
            ]
            out_sb = consts.tile([128, MT, OB, OS], F32)
            for ob in range(OB):
                for k in range(KT):
                    sl = slice(G2 + (ob * KT + k) * WKO,
                               G2 + (ob * KT + k + 1) * WKO)
                    for m in range(MT):
                        nc.tensor.matmul(
                            y_ps[m][ob],
                            lhsT=xall[:, k, m * 128:(m + 1) * 128],
                            rhs=wall[:, sl].rearrange("p (o j) -> p o j", j=NB),
                            start=(k == 0), stop=(k == KT - 1),
                        )
                for m in range(MT):
                    cb = bass.AP(
                        tensor=coef_sb[m].tensor, offset=coef_sb[m].offset,
                        ap=[list(coef_sb[m].ap[0]), [0, OS],
                            list(coef_sb[m].ap[1])],
                    )
                    tmp = work.tile([128, OS, NB], F32, tag="tmp")
                    nc.vector.tensor_tensor(
                        out=tmp, in0=y_ps[m][ob], in1=cb,
                        op=mybir.AluOpType.mult,
                    )
                    nc.vector.reduce_sum(
                        out=out_sb[:, m, ob, :], in_=tmp,
                        axis=mybir.AxisListType.X,
                    )
                    off = (m * OB + ob) * OS
                    nc.scalar.dma_start(out=out[:, off:off + OS],
                                        in_=out_sb[:, m, ob, :])

    nc.compile()
    return nc


def _get_nc():
    global _NC_CACHE
    if _NC_CACHE is None:
        _NC_CACHE = _build_nc()
    return _NC_CACHE


def _make_in_maps(x, idx_author, idx_citation, emb_author, emb_citation,
                  W1, b1, W2, W3, b3):
    f = np.float32
    x = np.asarray(x, dtype=f)
    W3r = np.asarray(W3, dtype=f).reshape(IN, OUT, NB)
    b3r = np.asarray(b3, dtype=f).reshape(IN, OUT)
    W1 = np.asarray(W1, dtype=f)

    # param-only folds: per-vocab gather tables G = emb @ W1half.T [VOCAB, KQ]
    gg = np.zeros((128, G2), f)
    gg[:VOCAB, :KQ] = np.asarray(emb_author, dtype=f) @ W1[:, :EMB].T
    gg[:VOCAB, KQ:] = np.asarray(emb_citation, dtype=f) @ W1[:, EMB:].T
    # packed f32 smalls [64, 4]: W2.T | b1
    sf = np.ascontiguousarray(np.concatenate([
        np.asarray(W2, dtype=f).T,
        np.asarray(b1, dtype=f).reshape(KQ, 1),
    ], axis=1))

    ia = np.asarray(idx_author).astype(ml_dtypes.bfloat16)
    ic = np.asarray(idx_citation).astype(ml_dtypes.bfloat16)
    ones_bs = np.ones((BS,), ml_dtypes.bfloat16)
    bw = np.stack([np.ones(VOCAB, f), -np.arange(VOCAB, dtype=f)]
                  ).astype(ml_dtypes.bfloat16)

    # per out-shard weight blocks, bias folded in, j innermost, os-block then
    # k packed, gather tables at the head
    wc_blocks = []
    for oj in range(Q_O):
        cols = slice(oj * OB * OS, (oj + 1) * OB * OS)
        blk = W3r[:, cols, :] + b3r[:, cols, None]       # [IN, OB*OS, NB]
        t = blk.reshape(KT, 128, OB, OS * NB).transpose(1, 2, 0, 3)
        wc_blocks.append(np.ascontiguousarray(np.concatenate(
            [gg, t.reshape(128, OB * KT * OS * NB)], axis=1)))

    # x.T per batch shard, k packed: [128, KT*BS]
    xt_shards = []
    for bi in range(P_B):
        xs = x[bi * BS:(bi + 1) * BS, :].T               # [IN, BS]
        xs = xs.reshape(KT, 128, BS).transpose(1, 0, 2)
        xt_shards.append(np.ascontiguousarray(xs.reshape(128, KT * BS)))

    in_maps = []
    for c in range(P_B * Q_O):
        bi, oj = c // Q_O, c % Q_O  # 4 batch shards x 2 out shards
        rows = slice(bi * BS, (bi + 1) * BS)
        sb16 = np.concatenate([
            np.stack([ia[rows], ones_bs]),
            np.stack([ic[rows], ones_bs]),
            bw,
        ], axis=1)
        in_maps.append({
            "xt": xt_shards[bi],
            "wc": wc_blocks[oj],
            "sf": sf,
            "sb": np.ascontiguousarray(sb16),
        })
    return in_maps


def kernel(x, idx_author, idx_citation, emb_author, emb_citation,
           W1, b1, W2, W3, b3):
    global LAST_RESULT
    _ensure_ntff_hook_module()
    nc = _get_nc()
    in_maps = _make_in_maps(x, idx_author, idx_citation, emb_author,
                            emb_citation, W1, b1, W2, W3, b3)
    res = run_bass_kernel_spmd(nc, in_maps, core_ids=list(range(P_B * Q_O)))
    LAST_RESULT = res
    out = np.empty((B, OUT), dtype=np.float32)
    ow = OB * OS
    for c in range(P_B * Q_O):
        bi, oj = c // Q_O, c % Q_O
        blk = res.results[c]["out"].reshape(128, MT, ow).transpose(1, 0, 2)
        out[bi * BS:(bi + 1) * BS, oj * ow:(oj + 1) * ow] = \
            blk.reshape(BS, ow)
    return out


# revision 28
# speedup vs baseline: 1.0209x; 1.0209x over previous
"""Trainium2 Bass kernel for a basis-customized linear layer.

Reference computation (B=1024, IN=OUT=512, EMB=64, KQ=64, NB=3, VOCAB=100):
    embs = concat(emb_author[idx_author], emb_citation[idx_citation])  # [B, 128]
    h    = tanh(embs @ W1.T + b1)                                      # [B, 64]
    coef = softmax(h @ W2.T)                                           # [B, 3]
    w    = (coef @ W3.T + b3).reshape(B, IN, OUT)
    out  = einsum('bi,bio->bo', x, w)                                  # [B, 512]

Key rewrites:
  (1) w[b] = sum_j coef[b,j]*W3j + b3r, so
      out = sum_j coef[:,j] * (x @ W3j) + x @ b3r
      -- 3 shared [512,512] matmuls + a per-sample weighted combine, instead
      of materializing the 1GB per-sample weight.
  (2) since softmax coefs sum to 1, the bias folds into every basis block:
      out = sum_j coef[:,j] * (x @ (W3j + b3r))
  (3) the embedding gather is a one-hot matmul (the one-hot comes from a
      DMA-broadcast idx compared against an iota), fused with W1 via the
      host-precomputed per-vocab table G = emb @ W1half.T (param-only fold).

Sharding over 8 cores: batch 2-way x out-column 4-way. Each core gets
x.T[:, batch_half] (1MB) and its 3 basis column-blocks (0.75MB), computes
coef for its 512 rows on-device, and writes a [512, 128] output block.

Data movement: all big loads ride ONE queue (same-queue transfers complete in
order) as (w_k, x_k) pairs so the k-th accumulation sweep starts as soon as
its pair lands; the per-m combines then stagger behind the last sweep.
"""

import numpy as np
import ml_dtypes

import concourse.bass as bass
import concourse.tile as tile
from concourse import bacc, mybir
from concourse.bass_utils import run_bass_kernel_spmd

# Problem dims (hardcoded per contract)
B, IN, OUT = 1024, 512, 512
EMB, KQ, NB, VOCAB = 64, 64, 3, 100
P_B, Q_O = 4, 2            # batch shards x out-col shards = 8 cores
BS = B // P_B              # 256 batch rows per core
OS = 128                   # out-col block width
OB = OUT // Q_O // OS      # 2 out-col blocks per core
KT = IN // 128             # 4 contraction tiles
MT = BS // 128             # 2 batch tiles per core

F32 = mybir.dt.float32
F32R = mybir.dt.float32r
BF16 = mybir.dt.bfloat16

SF32 = NB + 1              # packed f32 small-input columns (W2.T | b1)
SB16 = 2 * BS + VOCAB      # idx_author | idx_citation | -iota rows, bf16
G2 = 2 * KQ                # gather-table columns at the head of wc

LAST_RESULT = None         # BassKernelResults of the most recent run (for test.py)

_NC_CACHE = None


def _ensure_ntff_hook_module():
    """bass_utils imports antenv.axon_hooks when BASS_TRACE is set; the module
    is absent on this image. Provide a no-op shim so tracing degrades
    gracefully instead of crashing."""
    import sys, types
    if "antenv.axon_hooks" in sys.modules:
        return
    try:
        import antenv
        import antenv.axon_hooks  # noqa: F401
    except ImportError:
        mod = types.ModuleType("antenv.axon_hooks")
        state = {"hook": None}
        mod.set_axon_ntff_profile_hook = lambda h: state.__setitem__("hook", h)
        mod.get_axon_ntff_profile_hook = lambda: state["hook"]
        sys.modules["antenv.axon_hooks"] = mod
        try:
            antenv.axon_hooks = mod
        except Exception:
            pass


def _build_nc():
    nc = bacc.Bacc("TRN2", target_bir_lowering=False, debug=False,
                   num_devices=P_B * Q_O)

    xt = nc.dram_tensor("xt", [128, KT * BS], F32R, kind="ExternalInput")
    wc = nc.dram_tensor("wc", [128, G2 + KT * OB * NB * OS], F32R,
                        kind="ExternalInput")
    sf = nc.dram_tensor("sf", [KQ, SF32], F32, kind="ExternalInput")
    sb = nc.dram_tensor("sb", [2, SB16], BF16, kind="ExternalInput")
    out = nc.dram_tensor("out", [128, MT * OB * OS], F32,
                         kind="ExternalOutput")

    with tile.TileContext(nc) as tc:
        with (
            tc.tile_pool(name="consts", bufs=1) as consts,
            tc.tile_pool(name="work", bufs=3) as work,
            tc.tile_pool(name="ps_idx", bufs=2, space="PSUM") as ps_idx,
            tc.tile_pool(name="ps_pre", bufs=2, space="PSUM") as ps_pre,
            tc.tile_pool(name="ps_y", bufs=4, space="PSUM") as ps_y,
        ):
            # ---- small loads on idle queues ----
            sb_sb = consts.tile([2, SB16], BF16)
            nc.scalar.dma_start(out=sb_sb, in_=sb[:, :])
            sf_sb = consts.tile([KQ, SF32], F32)
            nc.scalar.dma_start(out=sf_sb, in_=sf[:, :])
            idxa_sb = sb_sb[:, 0:BS]
            idxc_sb = sb_sb[:, BS:2 * BS]
            bw_sb = sb_sb[:, 2 * BS:2 * BS + VOCAB]

            # ---- big loads: ONE queue, (w_k, x_k) pair order ----
            WK = OB * NB * OS
            xall = consts.tile([128, KT, BS], F32R)
            wall = consts.tile([128, G2 + KT * WK], F32R)
            xt_k = xt[:, :].rearrange("p (k n) -> p k n", k=KT)
            nc.sync.dma_start(out=wall[:, 0:G2 + WK],
                              in_=wc[:, 0:G2 + WK])
            nc.sync.dma_start(out=xall[:, 0, :], in_=xt_k[:, 0, :])
            for k in range(1, KT):
                sl = slice(G2 + k * WK, G2 + (k + 1) * WK)
                nc.sync.dma_start(out=wall[:, sl], in_=wc[:, sl])
                nc.sync.dma_start(out=xall[:, k, :], in_=xt_k[:, k, :])

            gat_sb = wall[0:VOCAB, 0:KQ]
            gct_sb = wall[0:VOCAB, KQ:G2]

            # b1 routed through ACT so Tanh's bias dep is same-engine
            b1_sb = consts.tile([KQ, 1], F32)
            nc.scalar.copy(out=b1_sb, in_=sf_sb[:, NB:NB + 1])
            w2r_sb = consts.tile([KQ, NB + 1], F32R)
            nc.vector.memset(w2r_sb.bitcast(mybir.dt.uint32), 0)
            nc.vector.tensor_copy(out=w2r_sb[:, 0:NB], in_=sf_sb[:, 0:NB])

            # ---- stage A: coef for all BS rows ----
            # one-hot via K=2 matmul: psum[v,b] = idx[b]*1 + 1*(-v), then ==0
            bca_ps = ps_idx.tile([VOCAB, BS], F32, tag="idx")
            nc.tensor.matmul(bca_ps, lhsT=bw_sb, rhs=idxa_sb, start=True, stop=True)
            oha_sb = consts.tile([VOCAB, BS], F32R)
            nc.vector.tensor_scalar(
                out=oha_sb, in0=bca_ps, scalar1=0.0, scalar2=None,
                op0=mybir.AluOpType.is_equal,
            )
            bcc_ps = ps_idx.tile([VOCAB, BS], F32, tag="idx")
            nc.tensor.matmul(bcc_ps, lhsT=bw_sb, rhs=idxc_sb, start=True, stop=True)
            ohc_sb = consts.tile([VOCAB, BS], F32R)
            nc.vector.tensor_scalar(
                out=ohc_sb, in0=bcc_ps, scalar1=0.0, scalar2=None,
                op0=mybir.AluOpType.is_equal,
            )

            # fused gather + W1: preact.T [KQ, BS]
            pre_ps = ps_pre.tile([KQ, BS], F32, tag="pre")
            nc.tensor.matmul(pre_ps, lhsT=gat_sb, rhs=oha_sb, start=True, stop=False)
            nc.tensor.matmul(pre_ps, lhsT=gct_sb, rhs=ohc_sb, start=False, stop=True)
            ht_sb = consts.tile([KQ, BS], F32R)
            nc.scalar.activation(
                out=ht_sb, in_=pre_ps, func=mybir.ActivationFunctionType.Tanh,
                bias=b1_sb, scale=1.0,
            )

            # softmax over NB bases, per 128-row tile
            coef_sb = []
            for m in range(MT):
                lg_ps = ps_pre.tile([128, NB + 1], F32, tag="pre")
                nc.tensor.matmul(
                    lg_ps, lhsT=ht_sb[:, m * 128:(m + 1) * 128], rhs=w2r_sb,
                    start=True, stop=True,
                )
                e_sb = work.tile([128, NB], F32, tag="e")
                nc.scalar.activation(
                    out=e_sb, in_=lg_ps[:, 0:NB],
                    func=mybir.ActivationFunctionType.Exp,
                )
                s_sb = work.tile([128, 1], F32, tag="s")
                nc.vector.reduce_sum(out=s_sb, in_=e_sb, axis=mybir.AxisListType.X)
                r_sb = work.tile([128, 1], F32, tag="r")
                nc.vector.reciprocal(out=r_sb, in_=s_sb)
                cf = consts.tile([128, NB], F32, name=f"coef{m}", tag=f"coef{m}")
                nc.vector.tensor_scalar(
                    out=cf, in0=e_sb, scalar1=r_sb, scalar2=None,
                    op0=mybir.AluOpType.mult,
                )
                coef_sb.append(cf)

            # ---- stage B: Y[m,ob][b, o, j], k-outer so sweeps chase loads
            y_ps = [
                [ps_y.tile([128, OS, NB], F32, tag="y", name=f"y{m}_{ob}")
                 for ob in range(OB)]
                for m in range(MT)
            ]
            for k in range(KT):
                for m in range(MT):
                    for ob in range(OB):
                        sl = slice(G2 + k * WK + ob * NB * OS,
                                   G2 + k * WK + (ob + 1) * NB * OS)
                        nc.tensor.matmul(
                            y_ps[m][ob],
                            lhsT=xall[:, k, m * 128:(m + 1) * 128],
                            rhs=wall[:, sl].rearrange("p (o j) -> p o j", j=NB),
                            start=(k == 0), stop=(k == KT - 1),
                        )

            # ---- combine: out[b,o] = sum_j coef[b,j] * Y[b,o,j] ----
            out_sb = consts.tile([128, MT, OB, OS], F32)
            for m in range(MT):
                cb = bass.AP(
                    tensor=coef_sb[m].tensor, offset=coef_sb[m].offset,
                    ap=[list(coef_sb[m].ap[0]), [0, OS], list(coef_sb[m].ap[1])],
                )
                for ob in range(OB):
                    tmp = work.tile([128, OS, NB], F32, tag="tmp")
                    nc.vector.tensor_tensor(
                        out=tmp, in0=y_ps[m][ob], in1=cb,
                        op=mybir.AluOpType.mult,
                    )
                    nc.vector.reduce_sum(
                        out=out_sb[:, m, ob, :], in_=tmp,
                        axis=mybir.AxisListType.X,
                    )
                    off = (m * OB + ob) * OS
                    nc.scalar.dma_start(out=out[:, off:off + OS],
                                        in_=out_sb[:, m, ob, :])

    nc.compile()
    return nc


def _get_nc():
    global _NC_CACHE
    if _NC_CACHE is None:
        _NC_CACHE = _build_nc()
    return _NC_CACHE


def _make_in_maps(x, idx_author, idx_citation, emb_author, emb_citation,
                  W1, b1, W2, W3, b3):
    f = np.float32
    x = np.asarray(x, dtype=f)
    W3r = np.asarray(W3, dtype=f).reshape(IN, OUT, NB)
    b3r = np.asarray(b3, dtype=f).reshape(IN, OUT)
    W1 = np.asarray(W1, dtype=f)

    # param-only folds: per-vocab gather tables G = emb @ W1half.T [VOCAB, KQ]
    gg = np.zeros((128, G2), f)
    gg[:VOCAB, :KQ] = np.asarray(emb_author, dtype=f) @ W1[:, :EMB].T
    gg[:VOCAB, KQ:] = np.asarray(emb_citation, dtype=f) @ W1[:, EMB:].T
    # packed f32 smalls [64, 4]: W2.T | b1
    sf = np.ascontiguousarray(np.concatenate([
        np.asarray(W2, dtype=f).T,
        np.asarray(b1, dtype=f).reshape(KQ, 1),
    ], axis=1))

    ia = np.asarray(idx_author).astype(ml_dtypes.bfloat16)
    ic = np.asarray(idx_citation).astype(ml_dtypes.bfloat16)
    ones_bs = np.ones((BS,), ml_dtypes.bfloat16)
    bw = np.stack([np.ones(VOCAB, f), -np.arange(VOCAB, dtype=f)]
                  ).astype(ml_dtypes.bfloat16)

    # per out-shard weight blocks, bias folded in, j innermost, os-block then
    # k packed, gather tables at the head
    wc_blocks = []
    for oj in range(Q_O):
        cols = slice(oj * OB * OS, (oj + 1) * OB * OS)
        blk = W3r[:, cols, :] + b3r[:, cols, None]       # [IN, OB*OS, NB]
        blk = blk.reshape(KT, 128, OB * OS * NB).transpose(1, 0, 2)
        wc_blocks.append(np.ascontiguousarray(np.concatenate(
            [gg, blk.reshape(128, KT * OB * OS * NB)], axis=1)))

    # x.T per batch shard, k packed: [128, KT*BS]
    xt_shards = []
    for bi in range(P_B):
        xs = x[bi * BS:(bi + 1) * BS, :].T               # [IN, BS]
        xs = xs.reshape(KT, 128, BS).transpose(1, 0, 2)
        xt_shards.append(np.ascontiguousarray(xs.reshape(128, KT * BS)))

    in_maps = []
    for c in range(P_B * Q_O):
        bi, oj = c // Q_O, c % Q_O  # 4 batch shards x 2 out shards
        rows = slice(bi * BS, (bi + 1) * BS)
        sb16 = np.concatenate([
            np.stack([ia[rows], ones_bs]),
            np.stack([ic[rows], ones_bs]),
            bw,
        ], axis=1)
        in_maps.append({
            "xt": xt_shards[bi],
            "wc": wc_blocks[oj],
            "sf": sf,
            "sb": np.ascontiguousarray(sb16),
        })
    return in_maps


def kernel(x, idx_author, idx_citation, emb_author, emb_citation,
           W1, b1, W2, W3, b3):
    global LAST_RESULT
    _ensure_ntff_hook_module()
    nc = _get_nc()
    in_maps = _make_in_maps(x, idx_author, idx_citation, emb_author,
                            emb_citation, W1, b1, W2, W3, b3)
    res = run_bass_kernel_spmd(nc, in_maps, core_ids=list(range(P_B * Q_O)))
    LAST_RESULT = res
    out = np.empty((B, OUT), dtype=np.float32)
    ow = OB * OS
    for c in range(P_B * Q_O):
        bi, oj = c // Q_O, c % Q_O
        blk = res.results[c]["out"].reshape(128, MT, ow).transpose(1, 0, 2)
        out[bi * BS:(bi + 1) * BS, oj * ow:(oj + 1) * ow] = \
            blk.reshape(BS, ow)
    return out
